# revision 1
# baseline (speedup 1.0000x reference)
"""Trainium2 Bass kernel for nn_NetLinkEvaluate (2-layer GCN + link decoder).

Strategy (8 NeuronCores, SPMD single program, per-core data):
  - Nodes sharded by range: core c owns dst rows [c*12500, (c+1)*12500).
  - Dense transforms (x@W1, z1@W2, z2@Wdec-halves) computed on the owning
    core in transposed layouts (no on-device transposes needed anywhere),
    shards AllGathered into full DRAM tables.
  - Edge aggregation: edges bucketed host-side by (owning core, 128-wide dst
    block), padded into 128-edge tiles. Per tile: indirect-DMA gather of the
    128 source rows from the table, scale by edge weight (ScalarE), build a
    one-hot selection matrix S[e, j] = (dstloc_e == j) (VectorE iota
    compare), and accumulate aggT += msg.T @ S into PSUM (TensorE).
    PSUM accumulation per dst block; no scatter hazards at all.
  - Decode: u/v = z2 @ Wdec halves packed in a [N, 4] table; per 128 decode
    edges two indirect gathers (src row, dst row) + one VectorE add.

All indices/structure are host-prepared; only values flow through engines.
"""
import math
import numpy as np

import concourse.bass as bass
import concourse.bacc as bacc
import concourse.mybir as mybir
import concourse.tile as tile

# Problem shapes (fixed by the task)
N = 100000
E = 1000000
PE = 200000
NFEAT = 128
NHID = 64

C = 8                       # cores
BLK = 128                   # dst block width

F32 = mybir.dt.float32
I32 = mybir.dt.int32


def _dims():
    NPC = N // C                       # nodes per core
    NBLK = math.ceil(NPC / BLK)        # dst blocks per core
    NPCP = NBLK * BLK                  # padded nodes per core
    NG = C * NPCP                      # padded global table rows
    NPE_C = PE // C                    # decode edges per core
    NTd = math.ceil(NPE_C / 128)       # decode tiles per core
    return NPC, NBLK, NPCP, NG, NPE_C, NTd


def _table_row(n, NPC, NPCP):
    return (n // NPC) * NPCP + (n % NPC)


def _preprocess(x, edge_index, edge_weight, pos_edge_index, W1, W2, Wdec):
    """Build per-core input maps + the shared tile structure (T_b envelope)."""
    NPC, NBLK, NPCP, NG, NPE_C, NTd = _dims()

    src = np.asarray(edge_index[0], dtype=np.int64)
    dst = np.asarray(edge_index[1], dtype=np.int64)
    w = np.asarray(edge_weight, dtype=np.float32)

    core_of = dst // NPC
    dloc = dst - core_of * NPC
    blk = dloc // BLK
    jloc = (dloc % BLK).astype(np.float32)
    trow = _table_row(src, NPC, NPCP).astype(np.int32)

    # per (core, block) edge counts -> envelope tile counts
    cnt = np.zeros((C, NBLK), dtype=np.int64)
    np.add.at(cnt, (core_of, blk), 1)
    T_b = np.maximum(1, np.ceil(cnt.max(axis=0) / 128).astype(np.int64))
    tile_base = np.concatenate([[0], np.cumsum(T_b)])
    NT = int(tile_base[-1])

    eidx = np.zeros((C, 128, NT), dtype=np.int32)
    edst = np.full((C, 128, NT), 999.0, dtype=np.float32)
    ew = np.zeros((C, 128, NT), dtype=np.float32)

    order = np.lexsort((src, blk, core_of))
    so_core, so_blk = core_of[order], blk[order]
    so_trow, so_jloc, so_w = trow[order], jloc[order], w[order]
    # position of each edge within its (core, block) bucket
    grp = so_core * NBLK + so_blk
    grp_starts = np.searchsorted(grp, np.arange(C * NBLK), side="left")
    pos_in_grp = np.arange(len(order)) - grp_starts[grp]
    slot_tile = tile_base[so_blk] + pos_in_grp // 128
    slot_part = pos_in_grp % 128
    eidx[so_core, slot_part, slot_tile] = so_trow
    edst[so_core, slot_part, slot_tile] = so_jloc
    ew[so_core, slot_part, slot_tile] = so_w

    # decode edges
    ps = np.asarray(pos_edge_index[0], dtype=np.int64)
    pd = np.asarray(pos_edge_index[1], dtype=np.int64)
    dsi = np.zeros((C, 128, NTd), dtype=np.int32)
    ddi = np.zeros((C, 128, NTd), dtype=np.int32)
    k = np.arange(NPE_C)
    tt, pp = k // 128, k % 128
    for c in range(C):
        ks = c * NPE_C + k
        dsi[c, pp, tt] = _table_row(ps[ks], NPC, NPCP).astype(np.int32)
        ddi[c, pp, tt] = _table_row(pd[ks], NPC, NPCP).astype(np.int32)

    # transposed, zero-padded x shards
    x = np.asarray(x, dtype=np.float32)
    xT = np.zeros((C, NFEAT, NPCP), dtype=np.float32)
    for c in range(C):
        xT[c, :, :NPC] = x[c * NPC:(c + 1) * NPC, :].T

    W1 = np.asarray(W1, dtype=np.float32)
    W2 = np.asarray(W2, dtype=np.float32)
    Wdec = np.asarray(Wdec, dtype=np.float32)
    AB = np.concatenate([Wdec[:, :NHID].T, Wdec[:, NHID:].T], axis=1)  # [H, 4]
    iota2 = np.tile(np.arange(128, dtype=np.float32), (128, 1))

    in_maps = []
    for c in range(C):
        in_maps.append({
            "xT": xT[c],
            "W1": W1,
            "W2s": W2,
            "AB": AB.astype(np.float32),
            "iota2": iota2,
            "eidx": eidx[c],
            "edst": edst[c],
            "ew": ew[c],
            "dsi": dsi[c],
            "ddi": ddi[c],
        })
    return in_maps, T_b.tolist(), tile_base, NT


def _build(NT, T_b):
    """Build the SPMD Bass program (identical across cores)."""
    NPC, NBLK, NPCP, NG, NPE_C, NTd = _dims()
    H = NHID

    nc = bacc.Bacc("TRN2", target_bir_lowering=False, debug=False, num_devices=C)

    xT_t = nc.dram_tensor("xT", [NFEAT, NPCP], F32, kind="ExternalInput")
    W1_t = nc.dram_tensor("W1", [NFEAT, H], F32, kind="ExternalInput")
    W2_t = nc.dram_tensor("W2s", [H, H], F32, kind="ExternalInput")
    AB_t = nc.dram_tensor("AB", [H, 4], F32, kind="ExternalInput")
    io_t = nc.dram_tensor("iota2", [128, 128], F32, kind="ExternalInput")
    eidx_t = nc.dram_tensor("eidx", [128, NT], I32, kind="ExternalInput")
    edst_t = nc.dram_tensor("edst", [128, NT], F32, kind="ExternalInput")
    ew_t = nc.dram_tensor("ew", [128, NT], F32, kind="ExternalInput")
    dsi_t = nc.dram_tensor("dsi", [128, NTd], I32, kind="ExternalInput")
    ddi_t = nc.dram_tensor("ddi", [128, NTd], I32, kind="ExternalInput")
    dec_t = nc.dram_tensor("dec", [NTd * 128, 2], F32, kind="ExternalOutput")

    groups = [list(range(C))]

    with tile.TileContext(nc) as tc:
        with tc.tile_pool(name="dram", bufs=1, space="DRAM") as dram, \
             tc.tile_pool(name="const", bufs=1) as cst, \
             tc.tile_pool(name="zbuf", bufs=1) as zb, \
             tc.tile_pool(name="xt", bufs=3) as xtp, \
             tc.tile_pool(name="gath", bufs=6) as gp, \
             tc.tile_pool(name="msg", bufs=4) as mp, \
             tc.tile_pool(name="sel", bufs=4) as sp, \
             tc.tile_pool(name="cpo", bufs=3) as cpo, \
             tc.tile_pool(name="uvt", bufs=6) as uvp, \
             tc.tile_pool(name="psA", bufs=2, space="PSUM") as psA, \
             tc.tile_pool(name="psB", bufs=2, space="PSUM") as psB:

            xw1_sh = dram.tile([NPCP, H], F32)
            xw1_full = dram.tile([NG, H], F32)
            zw2_sh = dram.tile([NPCP, H], F32)
            zw2_full = dram.tile([NG, H], F32)
            uv_sh = dram.tile([NPCP, 4], F32)
            uv_full = dram.tile([NG, 4], F32)

            W1s = cst.tile([NFEAT, H], F32)
            nc.sync.dma_start(out=W1s[:], in_=W1_t.ap()[:])
            W2s = cst.tile([H, H], F32)
            nc.sync.dma_start(out=W2s[:], in_=W2_t.ap()[:])
            ABs = cst.tile([H, 4], F32)
            nc.sync.dma_start(out=ABs[:], in_=AB_t.ap()[:])
            iotas = cst.tile([128, 128], F32)
            nc.sync.dma_start(out=iotas[:], in_=io_t.ap()[:])
            eidxs = cst.tile([128, NT], I32)
            nc.sync.dma_start(out=eidxs[:], in_=eidx_t.ap()[:])
            edsts = cst.tile([128, NT], F32)
            nc.sync.dma_start(out=edsts[:], in_=edst_t.ap()[:])
            ews = cst.tile([128, NT], F32)
            nc.sync.dma_start(out=ews[:], in_=ew_t.ap()[:])
            dsis = cst.tile([128, NTd], I32)
            nc.sync.dma_start(out=dsis[:], in_=dsi_t.ap()[:])
            ddis = cst.tile([128, NTd], I32)
            nc.sync.dma_start(out=ddis[:], in_=ddi_t.ap()[:])

            z1T = zb.tile([H, NPCP], F32)
            z2T = zb.tile([H, NPCP], F32)

            # ---- Phase A: xw1 = x @ W1 (shard), transposed-free layouts ----
            for b in range(NBLK):
                xt = xtp.tile([NFEAT, 128], F32, tag="xt")
                nc.sync.dma_start(out=xt[:], in_=xT_t.ap()[:, b * 128:(b + 1) * 128])
                ps = psA.tile([128, H], F32, tag="psA")
                nc.tensor.matmul(ps[:], xt[:], W1s[:], start=True, stop=True)
                cp = cpo.tile([128, H], F32, tag="cpo")
                nc.vector.tensor_copy(cp[:], ps[:])
                nc.sync.dma_start(out=xw1_sh[b * 128:(b + 1) * 128, :], in_=cp[:])

            nc.gpsimd.collective_compute(
                "AllGather", mybir.AluOpType.bypass, replica_groups=groups,
                ins=[xw1_sh.opt()], outs=[xw1_full.opt()])

            # ---- Phases B/D: edge aggregation ----
            def agg_layer(table_full, zT, relu):
                for b in range(NBLK):
                    ps = psB.tile([H, 128], F32, tag="psB")
                    t0, t1 = int(sum(T_b[:b])), int(sum(T_b[:b + 1]))
                    for t in range(t0, t1):
                        g = gp.tile([128, H], F32, tag="g")
                        nc.gpsimd.indirect_dma_start(
                            out=g[:], out_offset=None,
                            in_=table_full[:],
                            in_offset=bass.IndirectOffsetOnAxis(
                                ap=eidxs[:, t:t + 1], axis=0))
                        ms = mp.tile([128, H], F32, tag="ms")
                        nc.scalar.activation(
                            ms[:], g[:], mybir.ActivationFunctionType.Copy,
                            scale=ews[:, t:t + 1])
                        Sm = sp.tile([128, 128], F32, tag="Sm")
                        nc.vector.tensor_tensor(
                            out=Sm[:], in0=iotas[:],
                            in1=edsts[:, t:t + 1].to_broadcast([128, 128]),
                            op=mybir.AluOpType.is_equal)
                        nc.tensor.matmul(ps[:], ms[:], Sm[:],
                                         start=(t == t0), stop=(t == t1 - 1))
                    if relu:
                        nc.scalar.activation(
                            zT[:, b * 128:(b + 1) * 128], ps[:],
                            mybir.ActivationFunctionType.Relu)
                    else:
                        nc.vector.tensor_copy(zT[:, b * 128:(b + 1) * 128], ps[:])

            agg_layer(xw1_full, z1T, relu=True)

            # ---- Phase C: zw2 = z1 @ W2 ----
            for b in range(NBLK):
                ps = psA.tile([128, H], F32, tag="psA")
                nc.tensor.matmul(ps[:], z1T[:, b * 128:(b + 1) * 128], W2s[:],
                                 start=True, stop=True)
                cp = cpo.tile([128, H], F32, tag="cpo")
                nc.vector.tensor_copy(cp[:], ps[:])
                nc.sync.dma_start(out=zw2_sh[b * 128:(b + 1) * 128, :], in_=cp[:])

            nc.gpsimd.collective_compute(
                "AllGather", mybir.AluOpType.bypass, replica_groups=groups,
                ins=[zw2_sh.opt()], outs=[zw2_full.opt()])

            agg_layer(zw2_full, z2T, relu=False)

            # ---- Phase E: uv = z2 @ [A|B] ----
            for b in range(NBLK):
                ps = psA.tile([128, 4], F32, tag="psE")
                nc.tensor.matmul(ps[:], z2T[:, b * 128:(b + 1) * 128], ABs[:],
                                 start=True, stop=True)
                cp = cpo.tile([128, 4], F32, tag="cpoE")
                nc.vector.tensor_copy(cp[:], ps[:])
                nc.sync.dma_start(out=uv_sh[b * 128:(b + 1) * 128, :], in_=cp[:])

            nc.gpsimd.collective_compute(
                "AllGather", mybir.AluOpType.bypass, replica_groups=groups,
                ins=[uv_sh.opt()], outs=[uv_full.opt()])

            # ---- Phase F: decode ----
            outb = zb.tile([128, NTd * 2], F32)
            for t in range(NTd):
                uvt = uvp.tile([128, 8], F32, tag="uvt")
                nc.gpsimd.indirect_dma_start(
                    out=uvt[:, 0:4], out_offset=None, in_=uv_full[:],
                    in_offset=bass.IndirectOffsetOnAxis(ap=dsis[:, t:t + 1], axis=0))
                nc.gpsimd.indirect_dma_start(
                    out=uvt[:, 4:8], out_offset=None, in_=uv_full[:],
                    in_offset=bass.IndirectOffsetOnAxis(ap=ddis[:, t:t + 1], axis=0))
                nc.vector.tensor_add(
                    out=outb[:, t * 2:(t + 1) * 2],
                    in0=uvt[:, 0:2], in1=uvt[:, 6:8])
            nc.sync.dma_start(
                out=dec_t.ap().rearrange("(t p) c -> p t c", p=128),
                in_=outb[:].rearrange("p (t c) -> p t c", c=2))

    nc.compile()
    return nc


def kernel(x, edge_index, edge_weight, pos_edge_index, W1, W2, Wdec):
    from concourse import bass_utils
    NPC, NBLK, NPCP, NG, NPE_C, NTd = _dims()
    in_maps, T_b, tile_base, NT = _preprocess(
        x, edge_index, edge_weight, pos_edge_index, W1, W2, Wdec)
    nc = _build(NT, T_b)
    res = bass_utils.run_bass_kernel_spmd(nc, in_maps, core_ids=list(range(C)))
    out = np.concatenate(
        [res.results[c]["dec"][:NPE_C] for c in range(C)], axis=0)
    return out.astype(np.float32)
